# revision 46
# baseline (speedup 1.0000x reference)
"""Trainium2 Bass kernel for nn_Detection (nms_detection).

Strategy: data-parallel over batch (one batch per NeuronCore, 8 cores).
Host does pure layout prep: v is cast to bf16 and pre-transposed per batch to
vT [512, 8192] so the device matmul (contraction over D=512) needs no on-chip
transpose; rois / the label-selected pre_score column / gt-derived per-box
stats / one-hot / biases are packed into one pre-replicated [128, 556] f32
tensor loaded with a single DMA.

Device per core (measured ~28 us/iter steady-state, bf16 DMA roofline ~23.5):
  - PE: logits = vT.T @ W.T -> psum [128, 16*8] per quarter (cls 4 + reg 4
    cols); vT chunks are the stationary operand (bf16 => FWL), W.T [128, 8]
    the moving operand; ScalarE copies psum -> SBUF.
  - DVE/ACT: IoU vs 16 gt boxes as batched [128, 64(, 2), 16] ops; mask =
    (iou > 0.5) via u = inter/(areaG+areaR) > 1/3; best-gt gather via
    is_equal(u, umax) one-hot weighted sum; reciprocals as exp(-ln x) on
    ScalarE; smooth-L1 via m=min(|d|,1), sl1 = m*(|d|-0.5m); CE with clipped
    logits (log-sum-exp over the 4 classes).
  - Output: per-partition accumulators [128, 4]:
      col0 = sum lse*w, col1 = sum c_lab*w, col2 = sum mask, col3 = sum sl1*w
    where w = mask * pre_score[:, label].
Host: c_loss_b = (S0-S1)/(S2+1e-7); l1_b = S3/(BS*P);
      out = sum c_loss / (BS+1e-7) + sum l1.
"""

import os
import numpy as np
from contextlib import ExitStack

import concourse.bass as bass
import concourse.bacc as bacc
import concourse.tile as tile
from concourse import mybir
from concourse.bass_utils import run_bass_kernel_spmd

BS, P, G, D = 8, 8192, 16, 512
R, T = 128, 64          # P = T * R;  roi index p = t*128 + r
NQ = int(os.environ.get("DET_NQ", "4"))   # matmul column chunks
TQ = T // NQ            # 16 t-tiles per quarter
PQ = P // NQ            # 2048 roi columns per quarter
NCORES = 8

F32 = mybir.dt.float32
AF = mybir.ActivationFunctionType
OP = mybir.AluOpType
ONE_THIRD = float(np.float32(1.0) / np.float32(3.0))

USE_BF16 = os.environ.get("DET_BF16", "1") == "1"
if USE_BF16:
    import ml_dtypes
    DT_V = mybir.dt.bfloat16
    NP_V = ml_dtypes.bfloat16
else:
    DT_V = F32
    NP_V = np.float32


def _declare_io(nc):
    return dict(
        vt=nc.dram_tensor("vt", [D, P], DT_V, kind="ExternalInput").ap(),
        wt=nc.dram_tensor("wt", [D, 8], DT_V, kind="ExternalInput").ap(),
        smalls_d=nc.dram_tensor("smalls", [R, 556], F32, kind="ExternalInput").ap(),
        out_d=nc.dram_tensor("out_acc", [R, 4], F32, kind="ExternalOutput").ap(),
    )


def _body(nc, tc, pools, io):
    """Build the per-core program. All tensors below are per-core shards."""
    vt, wt, out_d = io["vt"], io["wt"], io["out_d"]

    pool, vpool, ppool = pools["main"], pools["vtp"], pools["psum"]

    # ---------- small inputs: one DMA, host pre-replicated / pre-packed ----
    # layout: [0:224] gt stats (14 rows of 16), [224:228] onehot,
    #         [228:236] bias8, [236:492] rois [T,4], [492:556] pre_score sel
    smalls = pool.tile([R, 556], F32, bufs=2)
    nc.sync.dma_start(out=smalls, in_=io["smalls_d"])
    gt_rep = smalls[:, 0:224]
    oh_rep = smalls[:, 224:228]
    bias_rep = smalls[:, 228:236]
    rois = smalls[:, 236:492].rearrange("j (t k) -> j t k", t=T)
    psc = smalls[:, 492:556]
    wt_sb = pool.tile([R, 4, 8], DT_V, bufs=2)
    nc.sync.dma_start(out=wt_sb, in_=wt.rearrange("(dc j) c -> j dc c", j=R))

    def gt_row(k):
        # [R, G] replicated row k of gt stats -> broadcast over t
        return gt_rep[:, k * G:(k + 1) * G][:, None, :].broadcast_to([R, T, G])

    ones1 = pool.tile([1, R], F32)
    nc.vector.memset(ones1, 1.0)

    # Pin the ACT table set (natural_log_exp contains Ln/Exp/Relu/Abs/Copy)
    # with a tiny op so the one-time table load overlaps the input DMAs.
    eps8 = pool.tile([R, 1], F32)
    nc.vector.memset(eps8, 1e-8)
    warm = pool.tile([R, 1], F32)
    nc.scalar.activation(out=warm, in_=eps8, func=AF.Ln)

    # ---------- matmul: logits[p, c] for c in 0..7 (cls 0:4, reg 4:8) ----------
    mm = pool.tile([R, T, 8], F32)
    for q in range(NQ):
        vq = vpool.tile([R, 4, PQ], DT_V, tag="vq")
        nc.sync.dma_start(
            out=vq, in_=vt[:, q * PQ:(q + 1) * PQ].rearrange("(dc j) p -> j dc p", j=R)
        )
        ps = ppool.tile([R, TQ * 8], F32, tag="ps")
        for tl in range(TQ):
            for dc in range(4):
                nc.tensor.matmul(
                    ps[:, tl * 8:(tl + 1) * 8],
                    lhsT=vq[:, dc, tl * R:(tl + 1) * R],
                    rhs=wt_sb[:, dc, :],
                    start=(dc == 0),
                    stop=False,
                )
            # bias via rank-1 accumulation: ones.T @ bias_row (K=1)
            nc.tensor.matmul(
                ps[:, tl * 8:(tl + 1) * 8],
                lhsT=ones1,
                rhs=bias_rep[0:1, :],
                start=False,
                stop=True,
            )
        nc.scalar.copy(out=mm[:, q * TQ:(q + 1) * TQ, :], in_=ps)

    # ---------- roi-derived stats ----------
    rc2 = pool.tile([R, T, 2], F32)       # x0+x1, y0+y1
    nc.vector.tensor_tensor(out=rc2, in0=rois[:, :, 0:2], in1=rois[:, :, 2:4], op=OP.add)
    rwh2 = pool.tile([R, T, 2], F32)      # x1-x0, y1-y0 (full widths)
    nc.vector.tensor_tensor(out=rwh2, in0=rois[:, :, 2:4], in1=rois[:, :, 0:2], op=OP.subtract)
    area_r = pool.tile([R, T], F32)
    nc.vector.tensor_tensor(out=area_r, in0=rwh2[:, :, 0], in1=rwh2[:, :, 1], op=OP.mult)
    # rmul = (1/(rw+1e-8), 1/(rh+1e-8), same, same) where rw = 0.5*(x1-x0)
    # reciprocal via exp(-ln(x)): ACT Reciprocal is banned, DVE reciprocal slow
    rmul = pool.tile([R, T, 4], F32)
    rwh_in = bass.AP(tensor=rwh2.tensor, offset=rwh2.offset,
                     ap=[rwh2.ap[0], [2, T], [0, 2], [1, 2]])
    rmul_out = bass.AP(tensor=rmul.tensor, offset=rmul.offset,
                       ap=[rmul.ap[0], [4, T], [2, 2], [1, 2]])
    nc.scalar.activation(out=rmul_out, in_=rwh_in, func=AF.Ln,
                         bias=eps8, scale=0.5)
    nc.scalar.activation(out=rmul, in_=rmul, func=AF.Exp, scale=-1.0)

    # ---------- IoU pipeline, [R, T, G] ----------
    def tt(name, in0, in1, op, shape=(R, T, G), engine=None):
        t_ = pool.tile(list(shape), F32, tag=name)
        (engine or nc.vector).tensor_tensor(out=t_, in0=in0, in1=in1, op=op)
        return t_

    # lt/rb/w fused over both axes: [R, T, 2, G] (xy packed as middle dim)
    def gt_rows2(k):
        # rows k, k+1 of gt stats -> [R, T, 2, G] broadcast over t
        return gt_rep[:, k * G:(k + 2) * G].rearrange("j (x g) -> j x g", x=2)[
            :, None, :, :].broadcast_to([R, T, 2, G])

    def roi_cols2(k):
        return rois[:, :, k:k + 2][:, :, :, None].broadcast_to([R, T, 2, G])

    # s_ab first so ACT's ln/exp (reciprocal) overlaps DVE's lt/rb/w work
    s_ab = tt("s_ab", gt_row(4), area_r[:, :, None].broadcast_to([R, T, G]), OP.add)
    rs = pool.tile([R, T, G], F32)
    nc.scalar.activation(out=rs, in_=s_ab, func=AF.Ln)
    nc.scalar.activation(out=rs, in_=rs, func=AF.Exp, scale=-1.0)
    lt = tt("lt", gt_rows2(0), roi_cols2(0), OP.max, shape=(R, T, 2, G))
    rb = tt("rb", gt_rows2(2), roi_cols2(2), OP.min, shape=(R, T, 2, G))
    w2 = tt("w2", rb, lt, OP.subtract, shape=(R, T, 2, G))
    nc.vector.tensor_scalar(out=w2, in0=w2, scalar1=0.0, scalar2=None, op0=OP.max)
    inter = tt("inter", w2[:, :, 0, :], w2[:, :, 1, :], OP.mult)
    u = tt("u", inter, rs, OP.mult)
    umax = pool.tile([R, T], F32)
    nc.vector.tensor_reduce(out=umax, in_=u, axis=mybir.AxisListType.X, op=OP.max)
    mask = pool.tile([R, T], F32)
    nc.vector.tensor_scalar(out=mask, in0=umax, scalar1=ONE_THIRD, scalar2=None,
                            op0=OP.is_gt)
    # one-hot of max gt per roi; ties only matter when masked out
    ohg = tt("ohg", u, umax[:, :, None].broadcast_to([R, T, G]), OP.is_equal)
    # gather gt stats (gxc, gyc, gwc, ghc) at the argmax via weighted sum
    prod = pool.tile([R, T, 4, G], F32)
    nc.vector.tensor_tensor(
        out=prod,
        in0=ohg[:, :, None, :].broadcast_to([R, T, 4, G]),
        in1=gt_rep[:, 5 * G:9 * G].rearrange("j (k g) -> j k g", k=4)[:, None, :, :]
            .broadcast_to([R, T, 4, G]),
        op=OP.mult,
    )
    gath = pool.tile([R, T, 4], F32)
    nc.vector.tensor_reduce(out=gath, in_=prod, axis=mybir.AxisListType.X, op=OP.add)

    # ---------- cls / ce (mm already includes the biases via the PE) -------
    cclip = pool.tile([R, T, 4], F32)
    nc.vector.tensor_scalar(out=cclip, in0=mm[:, :, 0:4], scalar1=1e-7,
                            scalar2=1.0 - 1e-7, op0=OP.max, op1=OP.min)
    e4 = pool.tile([R, T, 4], F32)
    nc.scalar.activation(out=e4, in_=cclip, func=AF.Exp)
    sume = pool.tile([R, T], F32)
    nc.vector.tensor_reduce(out=sume, in_=e4, axis=mybir.AxisListType.X, op=OP.add)
    lse = pool.tile([R, T], F32)
    nc.scalar.activation(out=lse, in_=sume, func=AF.Ln)
    # ps = pre_score[:, label] (host-gathered into psc [R, T])
    wm = pool.tile([R, T], F32)
    nc.vector.tensor_tensor(out=wm, in0=mask, in1=psc, op=OP.mult)
    ohwm = pool.tile([R, T, 4], F32)
    nc.vector.tensor_tensor(
        out=ohwm,
        in0=wm[:, :, None].broadcast_to([R, T, 4]),
        in1=oh_rep[:, None, :].broadcast_to([R, T, 4]),
        op=OP.mult,
    )

    acc = pool.tile([R, 4], F32)
    sc64 = pool.tile([R, T], F32)
    sc256 = pool.tile([R, T, 4], F32)
    # acc0 = sum lse * wm ; acc1 = sum c_lab * wm ; acc2 = sum mask
    nc.vector.scalar_tensor_tensor(out=sc64, in0=lse, scalar=1.0, in1=wm,
                                   op0=OP.mult, op1=OP.mult, accum_out=acc[:, 0:1])
    nc.vector.scalar_tensor_tensor(out=sc256, in0=cclip, scalar=1.0, in1=ohwm,
                                   op0=OP.mult, op1=OP.mult, accum_out=acc[:, 1:2])
    nc.vector.tensor_reduce(out=acc[:, 2:3], in_=mask, axis=mybir.AxisListType.X, op=OP.add)

    # ---------- regression / smooth-l1 ----------
    # gath[:, :, 0:2] -= roi center  (fused: (rc2 * -0.5) + gath)
    nc.vector.scalar_tensor_tensor(out=gath[:, :, 0:2], in0=rc2, scalar=-0.5,
                                   in1=gath[:, :, 0:2], op0=OP.mult, op1=OP.add)
    u4 = pool.tile([R, T, 4], F32)
    nc.vector.tensor_tensor(out=u4, in0=gath, in1=rmul, op=OP.mult)
    nc.scalar.activation(out=u4[:, :, 2:4], in_=u4[:, :, 2:4], func=AF.Ln)
    d4 = pool.tile([R, T, 4], F32)
    nc.vector.tensor_tensor(out=d4, in0=mm[:, :, 4:8], in1=u4, op=OP.subtract)
    a4 = pool.tile([R, T, 4], F32)
    nc.scalar.activation(out=a4, in_=d4, func=AF.Abs)
    m4 = pool.tile([R, T, 4], F32)
    nc.vector.tensor_scalar_min(m4, a4, 1.0)
    s4 = pool.tile([R, T, 4], F32)   # |d| - 0.5*min(|d|,1)
    nc.vector.scalar_tensor_tensor(out=s4, in0=m4, scalar=-0.5, in1=a4,
                                   op0=OP.mult, op1=OP.add)
    sl1 = pool.tile([R, T, 4], F32)
    nc.vector.tensor_tensor(out=sl1, in0=m4, in1=s4, op=OP.mult)
    nc.vector.scalar_tensor_tensor(
        out=sc256, in0=sl1, scalar=1.0,
        in1=wm[:, :, None].broadcast_to([R, T, 4]),
        op0=OP.mult, op1=OP.mult, accum_out=acc[:, 3:4])

    nc.sync.dma_start(out=out_d, in_=acc)


_PROGS = {}


def _get_program(repeats=1):
    if repeats not in _PROGS:
        nc = bacc.Bacc("TRN2", debug=False, enable_asserts=False)
        with ExitStack() as ctx:
            tc = ctx.enter_context(tile.TileContext(nc))
            io = _declare_io(nc)
            pools = dict(
                main=ctx.enter_context(tc.tile_pool(name="main", bufs=1)),
                vtp=ctx.enter_context(tc.tile_pool(name="vtp", bufs=2)),
                psum=ctx.enter_context(tc.tile_pool(name="psum", bufs=2, space="PSUM")),
            )
            for _ in range(repeats):
                _body(nc, tc, pools, io)
        nc.compile()
        _PROGS[repeats] = nc
    return _PROGS[repeats]


def make_in_maps(v, gt, rois, labels, pre_score, cls_w, cls_b, reg_w, reg_b):
    v = np.asarray(v, np.float32)
    gt = np.asarray(gt, np.float32)
    rois = np.asarray(rois, np.float32)
    labels = np.asarray(labels, np.float32)
    pre_score = np.asarray(pre_score, np.float32)
    W = np.concatenate([np.asarray(cls_w, np.float32), np.asarray(reg_w, np.float32)], 0)
    wt_full = np.ascontiguousarray(W.T).astype(NP_V)            # [512, 8]
    bias8 = np.concatenate([np.asarray(cls_b, np.float32),
                            np.asarray(reg_b, np.float32)]).astype(np.float32)
    lab = np.argmax(labels, axis=1)                             # [BS]

    vb = v.astype(NP_V).reshape(BS, P, D)
    in_maps = []
    for b in range(BS):
        vt_b = np.ascontiguousarray(vb[b].T)                    # [512, 8192]
        rois_p = np.ascontiguousarray(rois[b].reshape(T, R, 4).transpose(1, 0, 2))
        psc_p = np.ascontiguousarray(pre_score[b, :, lab[b]].reshape(T, R).T)
        g = gt[b]                                               # [16, 4]
        gtp = np.empty((10, G), np.float32)
        gtp[0], gtp[1], gtp[2], gtp[3] = g[:, 0], g[:, 1], g[:, 2], g[:, 3]
        gtp[4] = (g[:, 2] - g[:, 0]) * (g[:, 3] - g[:, 1])      # area_g
        gtp[5] = (g[:, 2] + g[:, 0]) / 2                        # gxc
        gtp[6] = (g[:, 3] + g[:, 1]) / 2                        # gyc
        gtp[7] = (g[:, 2] - g[:, 0]) / 2                        # gwc
        gtp[8] = (g[:, 3] - g[:, 1]) / 2                        # ghc
        gtp[9] = np.arange(G, dtype=np.float32)                 # iota over g
        # g-major (gxc, gyc, gwc, ghc) table for the indirect gather
        gstat_t = np.ascontiguousarray(gtp[5:9].T)              # [16, 4]
        gtp = np.concatenate([gtp.reshape(-1), gstat_t.reshape(-1)])
        oh = np.zeros(4, np.float32)
        oh[lab[b]] = 1.0
        smalls = np.empty((R, 556), np.float32)
        smalls[:, 0:224] = gtp
        smalls[:, 224:228] = oh
        smalls[:, 228:236] = bias8
        smalls[:, 236:492] = rois_p.reshape(R, T * 4)
        smalls[:, 492:556] = psc_p
        in_maps.append(dict(vt=vt_b, wt=wt_full, smalls=smalls))
    return in_maps


def combine(accs):
    """accs: list of 8 arrays [128, 4] -> final scalar (np.float32, shape ())."""
    S = np.stack([a.astype(np.float64).sum(axis=0) for a in accs])  # [8, 4]
    c_losses = (S[:, 0] - S[:, 1]) / (S[:, 2] + 1e-7)
    l1 = S[:, 3] / (BS * P)
    total = c_losses.sum() / (BS + 1e-7) + l1.sum()
    return np.asarray(total, dtype=np.float32)


def kernel(**inputs):
    nc = _get_program()
    in_maps = make_in_maps(**inputs)
    res = run_bass_kernel_spmd(nc, in_maps, core_ids=list(range(NCORES)))
    return combine([r["out_acc"] for r in res.results])
